# revision 12
# baseline (speedup 1.0000x reference)
"""Multi-head attention kernel for Trainium2, sharded over 8 NeuronCores.

Problem: Q,K,V [4, 16, 2048, 128] fp32 -> softmax(Q K^T / sqrt(128)) V.

Sharding: the 4*16 = 64 (batch, head) pairs are split across 8 cores,
8 pairs per core (pure data parallelism, no collectives).

Per-core kernel (flash-attention style, S^T layout). The scalar engine
(exp) is the roofline for this shape (2*2048^2*128 exps/core at 1
elem/cycle/lane = 218us), with the PE (fp16 matmuls, 219us) right at it,
so the design splits the exp work ACT/DVE and keeps everything else off
those engines:

  - Q, K are loaded fp32, cast to fp16 on DVE, and transposed to
    [d, seq] layout with ONE xbar DMA-transpose per tensor (out[d,t,s] =
    in[s, (t d)]); no PE identity-matmul transposes, no PSUM staging.
  - V is cast on GPSIMD into V_aug [k, 129] fp16 tiles whose last column
    is 1.0 (the PV matmul then also produces softmax row sums for free).
  - S^T[k, q] tiles = Kt_tile^T @ Qt_chunk land in PSUM in k-groups of
    (6, 6, 4) tiles per 256-wide q-chunk (stp: 2 slots x 3 banks).
  - exp: the first NT-N_DVE k-tiles of each group go to ACT
    (P^T = exp(S^T / sqrt(d)) as fp16, large-FD activates); the last
    N_DVE k-tiles are computed on the DVE as a Schraudolph bit-trick:
    int16(S^T * A + B) yields the bf16 bit pattern of exp(S^T/sqrt(d))
    to within +-3%, in ONE 1x tensor_scalar op (PSUM fp32 -> int16),
    reinterpreted as bf16 for the PV matmul. This offloads ~12-25% of
    the exp stream from the bottleneck engine at ~1e-2 worst-case
    output error (softmax averaging washes most of it out).
  - O_unnorm[q, 0:128] and row sums [q, 128] accumulate in PSUM over all
    k tiles via matmul(lhsT=P^T slice, rhs=V_aug).
  - Final normalize: O = O_unnorm * (1/sums) on the vector engine.

Scheduling: a global software pipeline over the (qc, group) stream (PV
matmuls trail the S^T/exp stream), with prep for later pairs (loads two
pairs ahead; casts and DMA-transposes one pair ahead) emitted into the
per-q-chunk gaps so the ACT/DVE/PE pipeline never drains at pair
boundaries. PSUM budget (8 banks): 2x3 S^T slots, 2 O accumulators.
"""

import os
import sys

for _p in ("/opt/trn_rl_repo",):
    if _p not in sys.path and os.path.isdir(_p):
        sys.path.insert(0, _p)

import numpy as np

import concourse.bass as bass
import concourse.bacc as bacc
import concourse.tile as tile
from concourse import mybir
from concourse.bass_utils import run_bass_kernel_spmd

F32 = mybir.dt.float32
F16 = mybir.dt.float16
BF16 = mybir.dt.bfloat16
I16 = mybir.dt.int16

B, H, S, D = 4, 16, 2048, 128
N_CORES = 8
PAIRS = (B * H) // N_CORES  # (b,h) pairs per core
P = 128  # partition dim / head dim / seq tile

# exp split: last N_DVE k-tiles (of NT) of every q-chunk go to the DVE
# via the Schraudolph bit-trick; the rest to ACT.
N_DVE = 2

# Schraudolph constants: bits16(s) = s*SCHRAU_A + SCHRAU_B, truncated to
# int16, reinterpreted bf16 ~= exp(s/sqrt(D)).
#   A = 128 * log2(e) / sqrt(D)
#   B = 128*127 - 128*0.0430 (centering) + 0.5 (trunc->round)
LOG2E = 1.4426950408889634
SCALE = float(1.0 / np.sqrt(D))
SCHRAU_A = float(128.0 * LOG2E * SCALE)
SCHRAU_B = float(128.0 * 127 - 128.0 * 0.0430 + 0.5)

_nc_cache = {}


def build_nc(pairs=PAIRS, seq=S, n_dve=None):
    """Build the per-core Bass program (SPMD: same program on all cores)."""
    if n_dve is None:
        n_dve = N_DVE if seq == S else (1 if seq >= 512 else 0)
    key = (pairs, seq, n_dve)
    if key in _nc_cache:
        return _nc_cache[key]

    NT = seq // P          # seq tiles (16)
    W = 256                # q-chunk width (2 psum O accumulators)
    QC = seq // W          # q chunks (8)
    NQT = W // P           # q subtiles per chunk (2)
    # k-tile groups per q chunk, sized to the stp PSUM slots (3 banks =
    # 6 k-tiles of W fp32)
    GK_MAX = 6
    groups = []
    k0 = 0
    while k0 < NT:
        g = min(GK_MAX, NT - k0)
        groups.append((k0, g))
        k0 += g
    DA = D + 1             # V augmented with a ones column
    assert 0 <= n_dve < min(GK_MAX, NT)
    # DVE-exp k-tiles: the last n_dve tiles (tail of the last group)
    DVE_T0 = NT - n_dve

    nc = bacc.Bacc("TRN2", target_bir_lowering=False, debug=False)
    Qd = nc.dram_tensor("Q", [pairs, seq, D], F32, kind="ExternalInput").ap()
    Kd = nc.dram_tensor("K", [pairs, seq, D], F32, kind="ExternalInput").ap()
    Vd = nc.dram_tensor("V", [pairs, seq, D], F32, kind="ExternalInput").ap()
    Od = nc.dram_tensor("O", [pairs, seq, D], F32, kind="ExternalOutput").ap()

    with tile.TileContext(nc) as tc:
        with (
            tc.tile_pool(name="consts", bufs=1) as consts,
            tc.tile_pool(name="ld32", bufs=3) as ld32_pool,
            tc.tile_pool(name="c16", bufs=2) as c16_pool,
            tc.tile_pool(name="tr", bufs=2) as tr_pool,
            tc.tile_pool(name="pt", bufs=4) as pt_pool,
            tc.tile_pool(name="ost", bufs=2) as ost_pool,
            tc.tile_pool(name="sm", bufs=8) as sm_pool,
            tc.tile_pool(name="st_ps", bufs=2, space="PSUM") as st_ps,
            tc.tile_pool(name="o_ps", bufs=2, space="PSUM") as o_ps,
        ):
            # explicit zero bias for exp: a float bias would become a
            # DMA-loaded const AP, entangling every ACTIVATE with a DMA
            # lane semaphore
            zbias = consts.tile([P, 1], F32)
            nc.vector.memset(zbias, 0.0)

            state = {}

            # Seq layout: row s of the pair lives at partition (s//2)%128,
            # tile 2*(s//256) + s%2. Each (partition, t2) covers 2
            # consecutive DRAM rows -> 1KB descriptors, halving the sync
            # sequencer's descriptor-dispatch load vs the naive [t,p,d]
            # pattern (512B) while keeping bursts short enough not to
            # stall engine SBUF access (8KB-per-partition bursts measurably
            # slow all engines by ~15%). Attention is permutation-invariant
            # along k, and the same permutation is applied to q on compute
            # and store, so results match.
            def emit_load(i, name, src_dram, eng=None):
                st = state.setdefault(i, {})
                st[name + "32"] = ld32_pool.tile(
                    [P, seq], F32, tag=name + "32", name=f"{name}32_{i}",
                    bufs=(4 if name == "Vb" else None),
                )
                (eng or nc.sync).dma_start(
                    out=st[name + "32"].rearrange(
                        "p (t2 c d) -> p t2 c d", c=2, d=P
                    ),
                    in_=src_dram.rearrange(
                        "(t2 p c) d -> p t2 c d", p=P, c=2
                    ),
                )

            def emit_cast(i, name):
                # fp32 -> fp16 on GPSIMD: the DVE queue must stay clear for
                # the Schraudolph exp ops (head-of-line blocking there stalls
                # the whole S^T pipeline via the stp slot rotation)
                st = state[i]
                st[name + "16"] = c16_pool.tile(
                    [P, seq], F16, tag=name + "16", name=f"{name}16_{i}"
                )
                nc.gpsimd.tensor_copy(out=st[name + "16"], in_=st[name + "32"])

            def emit_transpose(i, name):
                # one xbar DMA-transpose: [s, (t d)] -> [d, t, s]
                st = state[i]
                st[name + "t"] = tr_pool.tile(
                    [P, seq], F16, tag=name + "t", name=f"{name}t{i}"
                )
                nc.sync.dma_start_transpose(
                    st[name + "t"].rearrange("d (t s) -> d t s", s=P),
                    st[name + "16"][:, :],
                )

            VAUG_BUFS = 3

            def emit_cast_V(i, half):
                # gpsimd fp32->fp16 cast of half the V tiles (split so the
                # first PV of the next pair never waits on a 7us monolith).
                # The ones column is written once per pool buffer (pairs
                # 0..VAUG_BUFS-1) and never touched again: later pairs'
                # copies only overwrite the [0:D] columns.
                st = state[i]
                if half == 0:
                    st["Vaug"] = c16_pool.tile(
                        [P, NT * DA], F16, tag="Vaug", name=f"Vaug{i}",
                        bufs=VAUG_BUFS,
                    )
                vv = st["Vaug"].rearrange("p (t e) -> p t e", e=DA)
                t0, t1 = (0, NT // 2) if half == 0 else (NT // 2, NT)
                if i < VAUG_BUFS:
                    nc.gpsimd.memset(vv[:, t0:t1, D:DA], 1.0)
                nc.gpsimd.tensor_copy(
                    out=vv[:, t0:t1, 0:D],
                    in_=st["Vb32"].rearrange("p (t d) -> p t d", d=P)[:, t0:t1],
                )

            # gap_tasks: global gap index (pair*QC + qc) -> prep closures,
            # emitted right after that q-chunk completes (normalize). Prep
            # that would land before gap 0 is emitted upfront.
            gap_tasks = {}
            upfront = []

            def schedule(gap, fn):
                if gap < 0:
                    upfront.append((gap, len(upfront), fn))
                else:
                    gap_tasks.setdefault(gap, []).append(fn)

            for i in range(pairs):
                base = (i - 1) * QC  # gaps of the previous pair's main loop
                lbase = (i - 2) * QC  # loads go two pairs ahead
                g1 = min(1, max(0, QC - 1))
                g2 = min(2, max(0, QC - 2))
                g3 = min(3, max(0, QC - 1))
                g4 = min(4, max(0, QC - 1))
                schedule(lbase + 0, (lambda i=i: emit_load(i, "Kb", Kd[i])))
                schedule(lbase + g2, (lambda i=i: emit_load(i, "Qb", Qd[i])))
                schedule(lbase + g4, (lambda i=i: emit_load(i, "Vb", Vd[i])))
                schedule(base + 0, (lambda i=i: emit_cast(i, "Kb")))
                schedule(base + 0, (lambda i=i: emit_cast_V(i, 0)))
                schedule(base + g1, (lambda i=i: emit_cast(i, "Qb")))
                schedule(base + g2, (lambda i=i: emit_transpose(i, "Kb")))
                schedule(base + g2, (lambda i=i: emit_cast_V(i, 1)))
                schedule(base + g3, (lambda i=i: emit_transpose(i, "Qb")))

            for _, _, fn in sorted(upfront, key=lambda x: (x[0], x[1])):
                fn()

            # ---- global (qc, group) stream software pipeline ----
            qc_state = {}

            def finish_qc(i, qc):
                """Normalize + prep tasks + (if last qc) store for one q-chunk."""
                stq = qc_state.pop((i, qc))
                o_t = stq["o"]
                o_view = o_t[:, 0 : NQT * DA].rearrange("p (q e) -> p q e", e=DA)
                Ost = state[i]["Ost"]
                for qt in range(NQT):
                    t = qc * NQT + qt
                    rec = sm_pool.tile([P, 1], F32, tag="rec", name=f"rec{i}_{t}")
                    nc.vector.reciprocal(out=rec, in_=o_view[:, qt, D : D + 1])
                    nc.vector.tensor_scalar_mul(
                        Ost[:, t * P : (t + 1) * P], o_view[:, qt, 0:D], rec
                    )
                if qc == QC - 1:
                    nc.sync.dma_start(
                        out=Od[i].rearrange("(t2 p c) d -> p t2 c d", p=P, c=2),
                        in_=Ost.rearrange("p (t2 c d) -> p t2 c d", c=2, d=P),
                    )
                for fn in gap_tasks.pop(i * QC + qc, []):
                    fn()

            def emit_pv(ev, pt_f16, pt_bf16):
                """PV matmuls for one (pair, qc, group)."""
                i, qc, k0, gk = ev
                o_t = qc_state[(i, qc)]["o"]
                Vaug = state[i]["Vaug"]
                for j in range(gk):
                    kt = k0 + j
                    if kt < DVE_T0:
                        lhs_all = pt_f16
                    else:
                        lhs_all = pt_bf16
                    jj = j  # offset within the group tile
                    for qt in range(NQT):
                        nc.tensor.matmul(
                            o_t[:, qt * DA : (qt + 1) * DA],
                            lhsT=lhs_all[
                                :, jj * W + qt * P : jj * W + (qt + 1) * P
                            ],
                            rhs=Vaug[:, kt * DA : (kt + 1) * DA],
                            start=(kt == 0 and qt == 0),
                            stop=(kt == NT - 1 and qt == NQT - 1),
                        )
                if k0 + gk == NT:
                    finish_qc(i, qc)

            events = [
                (i, qc, k0, gk)
                for i in range(pairs)
                for qc in range(QC)
                for (k0, gk) in groups
            ]
            pvq = []
            for ev in events:
                i, qc, k0, gk = ev
                if k0 == 0:
                    if qc == 0:
                        state[i]["Ost"] = ost_pool.tile(
                            [P, seq], F32, tag="Ost", name=f"Ost{i}"
                        )
                    qc_state[(i, qc)] = {
                        "o": o_ps.tile([P, 512], F32, tag="o", name=f"o{i}_{qc}")
                    }
                Qt, Kt = state[i]["Qbt"], state[i]["Kbt"]
                stp = st_ps.tile(
                    [P, GK_MAX * W], F32, tag="st", name=f"st{i}_{qc}_{k0}"
                )
                for j in range(gk):
                    kt = k0 + j
                    nc.tensor.matmul(
                        stp[:, j * W : (j + 1) * W],
                        lhsT=Kt[:, kt * P : (kt + 1) * P],
                        rhs=Qt[:, qc * W : (qc + 1) * W],
                        start=True,
                        stop=True,
                    )
                # exp: ACT for k-tiles < DVE_T0, DVE bit-trick for the rest
                n_act = max(0, min(gk, DVE_T0 - k0))
                pt_f16 = pt_pool.tile(
                    [P, GK_MAX * W], F16, tag="pt", name=f"pt{i}_{qc}_{k0}"
                )
                pt_i16 = None
                if n_act > 0:
                    nc.scalar.activation(
                        out=pt_f16[:, 0 : n_act * W],
                        in_=stp[:, 0 : n_act * W],
                        func=mybir.ActivationFunctionType.Exp,
                        bias=zbias[:, 0:1],
                        scale=SCALE,
                    )
                if n_act < gk:
                    pt_i16 = pt_pool.tile(
                        [P, n_dve * W], I16, tag="pti", name=f"pti{i}_{qc}_{k0}",
                        bufs=4,
                    )
                    nc.vector.tensor_scalar(
                        out=pt_i16[:, 0 : (gk - n_act) * W],
                        in0=stp[:, n_act * W : gk * W],
                        scalar1=SCHRAU_A,
                        scalar2=SCHRAU_B,
                        op0=mybir.AluOpType.mult,
                        op1=mybir.AluOpType.add,
                    )
                # lhsT views for PV: fp16 part indexed by j, bf16 part by
                # (j - n_act) remapped via a shifted view
                if pt_i16 is not None:
                    ptb = pt_i16.bitcast(BF16)
                    # build a view aligned so that emit_pv can index both
                    # with the same j*W formula: prepend a dummy offset
                    pt_bf16 = _ShiftedView(ptb, n_act * W)
                else:
                    pt_bf16 = None
                pvq.append((ev, pt_f16, pt_bf16))
                if len(pvq) > 2:
                    emit_pv(*pvq.pop(0))
            while pvq:
                emit_pv(*pvq.pop(0))

    nc.compile()
    _nc_cache[key] = nc
    return nc


class _ShiftedView:
    """Index-shifted slicing proxy: view[:, a:b] -> base[:, a-ofs:b-ofs]."""

    def __init__(self, base, ofs):
        self.base = base
        self.ofs = ofs

    def __getitem__(self, idx):
        psl, fsl = idx
        return self.base[psl, fsl.start - self.ofs : fsl.stop - self.ofs]


def run(Q, K, V, trace=False):
    """Run on 8 cores; Q/K/V are full [B,H,S,D] fp32 arrays.

    Returns (output [B,H,S,D] fp32, BassKernelResults)."""
    Qf = np.ascontiguousarray(np.asarray(Q, dtype=np.float32).reshape(B * H, S, D))
    Kf = np.ascontiguousarray(np.asarray(K, dtype=np.float32).reshape(B * H, S, D))
    Vf = np.ascontiguousarray(np.asarray(V, dtype=np.float32).reshape(B * H, S, D))

    nc = build_nc()
    in_maps = [
        {
            "Q": Qf[c * PAIRS : (c + 1) * PAIRS],
            "K": Kf[c * PAIRS : (c + 1) * PAIRS],
            "V": Vf[c * PAIRS : (c + 1) * PAIRS],
        }
        for c in range(N_CORES)
    ]
    res = run_bass_kernel_spmd(nc, in_maps, list(range(N_CORES)), trace=trace)
    out = np.concatenate([res.results[c]["O"] for c in range(N_CORES)], axis=0)
    return out.reshape(B, H, S, D), res


def kernel(Q, K, V):
    # never trace in the grading path (the NTFF hook isn't available
    # outside our own test harness)
    prev = os.environ.get("BASS_NEVER_TRACE")
    os.environ["BASS_NEVER_TRACE"] = "1"
    try:
        out, _ = run(Q, K, V, trace=False)
    finally:
        if prev is None:
            os.environ.pop("BASS_NEVER_TRACE", None)
        else:
            os.environ["BASS_NEVER_TRACE"] = prev
    return out
